# revision 15
# baseline (speedup 1.0000x reference)
"""RBF (Gaussian) kernel Gram matrix on 8 Trainium2 NeuronCores.

out[i, j] = exp(-gamma * ||x_i - y_j||^2),  x, y: [8192, 256] fp32.

Strategy (data-parallel over rows of x; y replicated):
  - Rows of x are sharded across the 8 cores (1024 rows each).
  - Each core computes its [1024, 8192] stripe:
      psum = x_shard @ y.T                   (2 k-passes of 128, fp16 in, fp32 acc)
      E    = exp(2*gamma*psum - gamma*||x||^2)   (ACT engine; per-partition bias)
      out  = E * exp(-gamma*||y||^2)         (DVE tensor_mul against a host-
                                              broadcast fp16 [128, N] tile)
    which equals exp(-gamma*(||x||^2 + ||y||^2 - 2*x.y)).
  - Output is written fp16 (relative error ~5e-4 << 2e-2 budget), halving the
    dominant HBM traffic; the host upcasts to fp32 after the gather.
  - The steady state is ACT(exp)-bound at ~2.0us per [128, 2048] tile; the DMA
    order and a dummy activation (table preload) shave the pipeline head, and
    the last group is split so the tail drains faster.
"""

import numpy as np

GAMMA = 0.005
FULL_N = 8192
D = 256
N_CORES = 8
M_SHARD = FULL_N // N_CORES  # 1024 rows of x per core
P = 128
M_TILES = M_SHARD // P  # 8
GROUP = 2048  # columns of output produced per PSUM fill (4 banks)
BANK = 512  # fp32 columns per PSUM bank (one matmul's max free dim)
N_GROUPS = FULL_N // GROUP  # 4

_cache = {}


def _split_sync_waits(nc, maxw=1):
    """walrus codegen rejects instructions carrying more than ~2 sync waits
    ("Too many sync wait commands"). Tile can attach many (e.g. the tail
    drain waits on every semaphore; a matmul can wait on several DMA lanes).
    Hoist the excess onto wait-only EventSemaphore instructions inserted
    just before the offender on the same engine (engines execute their
    instructions in block order, so all waits still precede the op)."""
    import concourse.mybir as mybir

    n_new = 0
    for fn in nc.m.functions:
        for bb in fn.blocks:
            insts = bb.instructions
            if not any(
                i.sync_info is not None and len(i.sync_info.on_wait) > maxw
                for i in insts
            ):
                continue
            new = []
            for inst in insts:
                si = inst.sync_info
                if si is not None and len(si.on_wait) > maxw:
                    waits = list(si.on_wait)
                    for i in range(0, len(waits) - maxw, maxw):
                        ev = mybir.InstEventSemaphore(
                            name=f"wsplit_{n_new}", ins=[], outs=[]
                        )
                        n_new += 1
                        ev.engine = inst.engine
                        ev.sync_info = mybir.SyncInfo(
                            on_wait=waits[i : i + maxw], on_update=[]
                        )
                        new.append(ev)
                    si.on_wait = waits[len(waits) - maxw :]
                new.append(inst)
            bb.instructions = new


def _build():
    import concourse.bass as bass
    import concourse.mybir as mybir
    import concourse.tile as tile

    f32 = mybir.dt.float32
    f16 = mybir.dt.float16
    nc = bass.Bass("TRN2", target_bir_lowering=False, debug=False)
    # Inputs are host-packed into per-partition bundles so every DMA moves
    # large contiguous lines (fewer descriptors -> faster head):
    #   x2  [128, 8]    f32  -gamma*||x_i||^2, [m_tile, lane] layout
    #   b0  [128, 2560] f16  yt0[:,0:1024] | yt1[:,0:1024] | xt0[:,0:256] | xt1[:,0:256]
    #   b0b [128, 4096] f16  yt0[:,1024:2048] | yt1[:,1024:2048] | y2bc[:,0:2048]
    #   xtr [128, 1536] f16  xt0[:,256:1024] | xt1[:,256:1024]
    #   gb1..gb3 [128, 6144] f16  yt0[:,g] | yt1[:,g] | y2bc[:,g]  (2048 cols each)
    x2 = nc.dram_tensor("x2", [P, M_TILES], f32, kind="ExternalInput").ap()
    b0ad = nc.dram_tensor("b0a", [P, 1536], f16, kind="ExternalInput").ap()
    b0cd = nc.dram_tensor("b0c", [P, 1024], f16, kind="ExternalInput").ap()
    b0bd = nc.dram_tensor("b0b", [P, 4096], f16, kind="ExternalInput").ap()
    xtrd = nc.dram_tensor("xtr", [P, 1536], f16, kind="ExternalInput").ap()
    gbd = [
        nc.dram_tensor(f"gb{g}", [P, 3 * GROUP], f16, kind="ExternalInput").ap()
        for g in range(1, N_GROUPS)
    ]
    out = nc.dram_tensor("out", [M_SHARD, FULL_N], f16, kind="ExternalOutput").ap()

    XB = 2 * P  # x columns carried in b0 (m-tiles 0 and 1)

    with tile.TileContext(nc) as tc:
        with (
            tc.tile_pool(name="const", bufs=1) as cpool,
            tc.tile_pool(name="ep", bufs=3) as epool,
            tc.tile_pool(name="outp", bufs=5) as opool,
            tc.tile_pool(name="psum", bufs=2, space="PSUM") as ppool,
        ):
            # Dummy activation first: walrus puts the exp ACT_TABLE_LOAD right
            # before the first ACTIVATE in program order, so this hides the
            # ~1.3us table load under the input-DMA head.
            scratch = cpool.tile([P, BANK], f16, tag="scratch")
            sc32 = cpool.tile([P, 1], f32, tag="sc32")
            nc.any.memset(scratch, 0.0)
            nc.any.memset(sc32, 0.0)
            nc.scalar.activation(sc32, sc32, mybir.ActivationFunctionType.Exp)

            x2sb = cpool.tile([P, M_TILES], f32, tag="x2")
            b0a = cpool.tile([P, 1536], f16, tag="b0a")
            b0c = cpool.tile([P, 1024], f16, tag="b0c")
            b0b = cpool.tile([P, 4096], f16, tag="b0b")
            xtr = cpool.tile([P, 1536], f16, tag="xtr")
            gb1 = cpool.tile([P, 3 * GROUP], f16, tag="gb1")
            gb2 = cpool.tile([P, 3 * GROUP], f16, tag="gb2")
            gb3 = cpool.tile([P, 3 * GROUP], f16, tag="gb3")
            gb = [gb1, gb2, gb3]
            nc.sync.dma_start(out=b0a, in_=b0ad)
            nc.sync.dma_start(out=x2sb, in_=x2)
            nc.sync.dma_start(out=b0c, in_=b0cd)
            nc.sync.dma_start(out=b0b, in_=b0bd)
            nc.sync.dma_start(out=xtr, in_=xtrd)
            for i in range(N_GROUPS - 1):
                nc.sync.dma_start(out=gb[i], in_=gbd[i])

            def lhs_ap(t, d):
                if t < 2:
                    off = 1024 + d * 256 + t * P
                    return b0a[:, off : off + P]
                off = d * 768 + (t - 2) * P
                return xtr[:, off : off + P]

            def rhs_ap(g, d, b):
                if g == 0:
                    if b == 0:
                        return b0a[:, d * BANK : (d + 1) * BANK]
                    if b == 1:
                        return b0c[:, d * BANK : (d + 1) * BANK]
                    off = d * 1024 + (b - 2) * BANK
                    return b0b[:, off : off + BANK]
                off = d * GROUP + b * BANK
                return gb[g - 1][:, off : off + BANK]

            def y2_ap(g):
                if g == 0:
                    return b0b[:, 2048:4096]
                return gb[g - 1][:, 2 * GROUP : 3 * GROUP]

            # Warm the PE HAM clock-gate while the input bundles stream in so
            # the real stream starts at 2.4GHz, not 1.2.
            wt = ppool.tile([P, GROUP], f32, tag="ps")
            for w in range(4):
                nc.tensor.matmul(
                    wt[:, 0:BANK], scratch[:, 0:P], scratch,
                    start=True, stop=True,
                )

            def emit(t, g, flip, sets=((0, 1, 2, 3),)):
                msl = slice(t * P, (t + 1) * P)
                ps = ppool.tile([P, GROUP], f32, tag="ps")
                # weight-stationary order: one LDWEIGHTS per k-tile per
                # group; consecutive matmuls hit different PSUM banks so
                # fill/drain overlap. `flip` reverses the k-tile order so
                # adjacent groups of the same t share a boundary weight.
                # `split` halves the elementwise/DMA stages: the first EXP can
                # start after 4 matmuls (shorter head) and the last output
                # drains a half-tile (shorter tail).
                order = (1, 0) if flip else (0, 1)
                bank_sets = sets
                et = epool.tile([P, GROUP], f16, tag="et")
                ot = opool.tile([P, GROUP], f16, tag="ot")
                for bs in bank_sets:
                    for j, d in enumerate(order):
                        lhs = lhs_ap(t, d)
                        for b in bs:
                            bsl = slice(b * BANK, (b + 1) * BANK)
                            nc.tensor.matmul(
                                ps[:, bsl], lhs, rhs_ap(g, d, b),
                                start=(j == 0), stop=(j == 1),
                            )
                    psl = slice(bs[0] * BANK, (bs[-1] + 1) * BANK)
                    osl = slice(
                        g * GROUP + bs[0] * BANK, g * GROUP + (bs[-1] + 1) * BANK
                    )
                    # exp(2g*(x.y) - g*x2), fp16 out
                    nc.scalar.activation(
                        et[:, psl], ps[:, psl], mybir.ActivationFunctionType.Exp,
                        bias=x2sb[:, t : t + 1], scale=2.0 * GAMMA,
                    )
                    y2s = y2_ap(g)
                    nc.vector.tensor_mul(
                        ot[:, psl], et[:, psl],
                        y2s[:, bs[0] * BANK : (bs[-1] + 1) * BANK],
                    )
                    nc.sync.dma_start(out=out[msl, osl], in_=ot[:, psl])

            # Ramp phase: sweep all t over group 0 only — 8 iterations that
            # need just the first quarter of y, while groups 1-3 stream in.
            # t=0 is emitted in quarter/half pieces gated on the micro-bundles
            # b0a/b0c/b0b so the first EXP starts as early as possible.
            for t in range(M_TILES):
                emit(t, 0, flip=False,
                     sets=((0,), (1,), (2, 3)) if t == 0 else ((0, 1, 2, 3),))
            # Steady phase: per t, groups 1-3 with alternating k-tile order
            # (halves the weight switches on the PE). The final two tiles are
            # split so the endgame drains half-tiles.
            for t in range(M_TILES):
                for i, g in enumerate(range(1, N_GROUPS)):
                    last2 = t == M_TILES - 1 and g >= N_GROUPS - 2
                    emit(t, g, flip=(i % 2 == 1),
                         sets=((0, 1), (2, 3)) if last2 else ((0, 1, 2, 3),))

    _split_sync_waits(nc)
    return nc


def kernel(x: np.ndarray, y: np.ndarray) -> np.ndarray:
    from concourse import bass_utils

    x = np.asarray(x, dtype=np.float32)
    y = np.asarray(y, dtype=np.float32)

    if "nc" not in _cache:
        _cache["nc"] = _build()
    nc = _cache["nc"]

    yt = y.T.astype(np.float16)  # [256, 8192]
    yt0, yt1 = yt[0:P], yt[P : 2 * P]
    xt_full = x.T.astype(np.float16)  # [256, 8192]
    x2 = np.sum(x * x, axis=1)  # [8192]
    y2 = np.sum(y.astype(np.float64) * y.astype(np.float64), axis=1)
    y2bc = np.broadcast_to(np.exp(-GAMMA * y2).astype(np.float16), (P, FULL_N))

    XB = 2 * P
    HB = 2 * BANK
    # Shared (y-side) bundles, replicated across cores.
    b0b = np.ascontiguousarray(
        np.concatenate([yt0[:, HB:GROUP], yt1[:, HB:GROUP], y2bc[:, 0:GROUP]], axis=1)
    )
    gbs = [
        np.ascontiguousarray(
            np.concatenate(
                [
                    yt0[:, g * GROUP : (g + 1) * GROUP],
                    yt1[:, g * GROUP : (g + 1) * GROUP],
                    y2bc[:, g * GROUP : (g + 1) * GROUP],
                ],
                axis=1,
            )
        )
        for g in range(1, N_GROUPS)
    ]

    in_maps = []
    for c in range(N_CORES):
        cols = slice(c * M_SHARD, (c + 1) * M_SHARD)
        xtc = xt_full[:, cols]
        xt0c, xt1c = xtc[0:P], xtc[P : 2 * P]
        x2c = (-GAMMA * x2[cols]).astype(np.float32)
        imap = {
            "x2": np.ascontiguousarray(x2c.reshape(M_TILES, P).T),
            "b0a": np.ascontiguousarray(
                np.concatenate(
                    [yt0[:, 0:BANK], yt1[:, 0:BANK], xt0c[:, 0:XB], xt1c[:, 0:XB]],
                    axis=1,
                )
            ),
            "b0c": np.ascontiguousarray(
                np.concatenate([yt0[:, BANK:HB], yt1[:, BANK:HB]], axis=1)
            ),
            "b0b": b0b,
            "xtr": np.ascontiguousarray(
                np.concatenate([xt0c[:, XB:], xt1c[:, XB:]], axis=1)
            ),
        }
        for g in range(1, N_GROUPS):
            imap[f"gb{g}"] = gbs[g - 1]
        in_maps.append(imap)

    res = bass_utils.run_bass_kernel_spmd(
        nc, in_maps, core_ids=list(range(N_CORES))
    )
    _cache["last_result"] = res
    return np.concatenate(
        [res.results[c]["out"] for c in range(N_CORES)], axis=0
    ).astype(np.float32)
